# revision 6
# baseline (speedup 1.0000x reference)
"""AttentionPairBias TRN2 kernel — 8-core SPMD, query-row sharding, v2.

Differences vs v1 baseline:
  - z path in bf16: zT input is bf16 (half the DMA bytes), z-plane stationaries
    get fast-weight-load, z^2 computed in bf16 (2x DVE rate on half).
  - z-LN stats accumulated into persistent [q, k] buffers; the
    mean/var/rsqrt chain runs as a handful of [128, 1024] ops instead of
    per-16k micro-op chains.  zb normalization is one big [128, 16k] op.
  - zb raw stored [q, h, k] (bf16) so the per-head score injection streams
    contiguous k.
  - softmax normalization deferred past the AV matmul: exp (unnormalized,
    bf16) -> transpose -> AV -> scale o by 1/rowsum fused with the gate.
  - bk dropped (softmax-invariant).
  - v3: s-path weights/activations in bf16 too, shrinking SBUF so the zT
    prefetch pool coexists with phase A -- the 33 MB z stream starts at t=0
    instead of waiting ~55 us for phase-A pools to free.  zb finalize split
    into k-halves so phase-C injection starts before the full tail is done.
"""
import sys, os
sys.path.insert(0, "/opt/trn_rl_repo")
import numpy as np
import ml_dtypes

import concourse.bass as bass
import concourse.bacc as bacc
import concourse.mybir as mybir
import concourse.tile as tile
from concourse.bass_utils import run_bass_kernel_spmd

F32 = mybir.dt.float32
F32R = mybir.dt.float32r
BF16 = mybir.dt.bfloat16
AF = mybir.ActivationFunctionType
OP = mybir.AluOpType

B, N, H, HD, D, DZ = 1, 1024, 16, 32, 512, 128
NC = 8
NQ = N // NC          # 128 q rows per core
KC = 32               # k's per DMA chunk
EPS = 1e-5

_CACHED = None


def _build():
    nc = bacc.Bacc(None, target_bir_lowering=False)

    s_d = nc.dram_tensor("s_full", [N, D], F32, kind="ExternalInput")
    sq_d = nc.dram_tensor("s_q", [NQ, D], F32, kind="ExternalInput")
    zT_d = nc.dram_tensor("zT", [DZ, N, NQ], BF16, kind="ExternalInput")
    wq_d = nc.dram_tensor("Wq", [D, D], BF16, kind="ExternalInput")
    wk_d = nc.dram_tensor("Wk", [D, D], BF16, kind="ExternalInput")
    wv_d = nc.dram_tensor("Wv", [D, D], BF16, kind="ExternalInput")
    wg_d = nc.dram_tensor("Wg", [D, D], BF16, kind="ExternalInput")
    wo_d = nc.dram_tensor("Wo", [D, D], BF16, kind="ExternalInput")
    bq_d = nc.dram_tensor("bq", [D], F32, kind="ExternalInput")
    bv_d = nc.dram_tensor("bv", [D], F32, kind="ExternalInput")
    bg_d = nc.dram_tensor("bg", [D], F32, kind="ExternalInput")
    wext_d = nc.dram_tensor("Wext", [DZ, 18], BF16, kind="ExternalInput")
    id_d = nc.dram_tensor("ident", [128, 128], F32R, kind="ExternalInput")
    idb_d = nc.dram_tensor("identb", [128, 128], BF16, kind="ExternalInput")
    out_d = nc.dram_tensor("out", [NQ, D], F32, kind="ExternalOutput")

    with tile.TileContext(nc) as tc:
        with tc.tile_pool(name="const", bufs=1) as cpool, \
             tc.tile_pool(name="persist", bufs=1) as pp, \
             tc.tile_pool(name="ztp", bufs=8) as ztp:
            ident = cpool.tile([128, 128], F32R)
            nc.sync.dma_start(out=ident, in_=id_d[:, :])
            identb = cpool.tile([128, 128], BF16)
            nc.sync.dma_start(out=identb, in_=idb_d[:, :])
            wext = cpool.tile([DZ, 18], BF16)
            nc.sync.dma_start(out=wext, in_=wext_d[:, :])
            eps_t = cpool.tile([128, 1], F32)
            nc.vector.memset(eps_t, EPS)
            bq_t = cpool.tile([128, 4], F32)
            nc.sync.dma_start(out=bq_t, in_=bq_d[:].rearrange("(b p) -> p b", p=128))
            bg_rep = cpool.tile([128, D], F32)
            bg_ap = bg_d[:]
            nc.gpsimd.dma_start(
                out=bg_rep,
                in_=bass.AP(tensor=bg_ap.tensor, offset=bg_ap.offset,
                            ap=[[0, 128], [1, D]]),
            )
            bv_rep = cpool.tile([128, D], F32)
            bv_ap = bv_d[:]
            nc.gpsimd.dma_start(
                out=bv_rep,
                in_=bass.AP(tensor=bv_ap.tensor, offset=bv_ap.offset,
                            ap=[[0, 128], [1, D]]),
            )

            # ---------- persistent activation storage ----------
            slnT = [pp.tile([128, N], BF16, name=f"slnT{j}") for j in range(4)]
            sqT = pp.tile([128, 4, 128], BF16)        # (d%128, dtile, q)
            KT = [pp.tile([128, N], BF16, name=f"KT{b}") for b in range(4)]
            Vt = [pp.tile([128, D], BF16, name=f"V{t}") for t in range(8)]
            QT = [pp.tile([128, 128], BF16, name=f"QT{b}") for b in range(4)]
            G_sb = pp.tile([128, D], F32, name="G_sb")
            zb = pp.tile([128, H, N], BF16, name="zb")        # (q, h, k)
            muraw = pp.tile([128, N], F32, name="muraw")      # (q, k) sum_c z
            ssraw = pp.tile([128, N], F32, name="ssraw")      # (q, k) sum_c z^2
            alpha = pp.tile([128, N], BF16, name="alpha")
            rowsums = pp.tile([128, H], F32)

            with tc.tile_pool(name="z2B", bufs=2) as zp, \
                 tc.tile_pool(name="psB", bufs=2, space="PSUM") as psB, \
                 tc.tile_pool(name="psS", bufs=2, space="PSUM") as psSp:
              # ================= phase A: s path =================
              with tc.tile_pool(name="sA", bufs=3) as ap_, \
                   tc.tile_pool(name="wA", bufs=1) as wp, \
                   tc.tile_pool(name="psA", bufs=2, space="PSUM") as psA:
                wk = [wp.tile([128, D], BF16, name=f"wk{i}") for i in range(4)]
                wv = [wp.tile([128, D], BF16, name=f"wv{i}") for i in range(4)]
                wq = [wp.tile([128, D], BF16, name=f"wq{i}") for i in range(4)]
                wg = [wp.tile([128, D], BF16, name=f"wg{i}") for i in range(4)]
                for i in range(4):
                    sl = slice(i * 128, (i + 1) * 128)
                    nc.sync.dma_start(out=wk[i], in_=wk_d[sl, :])
                    nc.sync.dma_start(out=wv[i], in_=wv_d[sl, :])
                    nc.sync.dma_start(out=wq[i], in_=wq_d[sl, :])
                    nc.sync.dma_start(out=wg[i], in_=wg_d[sl, :])

                def layernorm_tile(src_ap, tag):
                    st = ap_.tile([128, D], F32, tag="st", name=f"st{tag}")
                    nc.sync.dma_start(out=st, in_=src_ap)
                    stats = ap_.tile([128, 6], F32, tag="stats", name=f"stats{tag}")
                    nc.vector.bn_stats(out=stats, in_=st)
                    mv = ap_.tile([128, 2], F32, tag="mv", name=f"mv{tag}")
                    nc.vector.bn_aggr(out=mv, in_=stats)
                    std = ap_.tile([128, 1], F32, tag="std", name=f"std{tag}")
                    nc.scalar.activation(out=std, in_=mv[:, 1:2], func=AF.Sqrt,
                                         bias=eps_t, scale=1.0)
                    rst = ap_.tile([128, 1], F32, tag="rst", name=f"rst{tag}")
                    nc.vector.reciprocal(rst, std)
                    sln = ap_.tile([128, D], BF16, tag="sln", name=f"sln{tag}")
                    nc.vector.scalar_tensor_tensor(
                        out=sln, in0=st, scalar=mv[:, 0:1],
                        in1=rst.to_broadcast((128, D)),
                        op0=OP.subtract, op1=OP.mult)
                    return sln

                # full-s LN + transpose into slnT
                for t in range(8):
                    sln = layernorm_tile(s_d[t * 128:(t + 1) * 128, :], f"s{t}")
                    ps = psA.tile([128, D], BF16, tag="trA")
                    for j in range(4):
                        nc.tensor.transpose(ps[:, j * 128:(j + 1) * 128],
                                            sln[:, j * 128:(j + 1) * 128], identb)
                    for j in range(4):
                        nc.vector.tensor_copy(slnT[j][:, t * 128:(t + 1) * 128],
                                              ps[:, j * 128:(j + 1) * 128])
                # q-block LN + transpose into sqT
                slnq = layernorm_tile(sq_d[:, :], "q")
                psq = psA.tile([128, D], BF16, tag="trA")
                for j in range(4):
                    nc.tensor.transpose(psq[:, j * 128:(j + 1) * 128],
                                        slnq[:, j * 128:(j + 1) * 128], identb)
                for j in range(4):
                    nc.vector.tensor_copy(sqT[:, j, :], psq[:, j * 128:(j + 1) * 128])

                # KT[b] = (sln @ Wk)^T  -> [hd(128b), tok]   (bk dropped:
                # a per-(q,h) additive constant is softmax-invariant)
                for b in range(4):
                    bs = slice(b * 128, (b + 1) * 128)
                    for half in range(2):
                        hs = slice(half * 512, (half + 1) * 512)
                        ps = psA.tile([128, 512], F32, tag="mmA")
                        for dt_ in range(4):
                            nc.tensor.matmul(ps, wk[dt_][:, bs], slnT[dt_][:, hs],
                                             start=(dt_ == 0), stop=(dt_ == 3))
                        nc.scalar.copy(KT[b][:, hs], ps)
                # V[t] = sln @ Wv + bv  (natural [tok, hd], bf16)
                for t in range(8):
                    ts = slice(t * 128, (t + 1) * 128)
                    ps = psA.tile([128, 512], F32, tag="mmA")
                    for dt_ in range(4):
                        nc.tensor.matmul(ps, slnT[dt_][:, ts], wv[dt_],
                                         start=(dt_ == 0), stop=(dt_ == 3))
                    nc.vector.tensor_add(Vt[t], ps, bv_rep)
                # QT[b] from the q-block
                for b in range(4):
                    bs = slice(b * 128, (b + 1) * 128)
                    psqt = psA.tile([128, 128], F32, tag="mmA")
                    for dt_ in range(4):
                        nc.tensor.matmul(psqt, wq[dt_][:, bs], sqT[:, dt_, :],
                                         start=(dt_ == 0), stop=(dt_ == 3))
                    nc.scalar.activation(out=QT[b], in_=psqt, func=AF.Identity,
                                         bias=bq_t[:, b:b + 1], scale=1.0)
                # G natural [q, D]
                psg = psA.tile([128, D], F32, tag="mmA")
                for dt_ in range(4):
                    nc.tensor.matmul(psg, sqT[:, dt_, :], wg[dt_],
                                     start=(dt_ == 0), stop=(dt_ == 3))
                gsum = ap_.tile([128, D], F32, tag="st", name="gsum")
                nc.vector.tensor_add(gsum, psg, bg_rep)
                nc.scalar.activation(out=G_sb, in_=gsum, func=AF.Sigmoid,
                                     bias=0.0, scale=1.0)

              # ================= phase B: z path =================
              # per k: LDW(z-plane) MM(17 cols: 16 heads + ones) into psB;
              #        LDW(z^2-plane) MM(2 cols: ones, 0) into psS.
              if True:
                for ci in range(N // KC):
                    zt = ztp.tile([128, KC, 128], BF16, tag="zt")
                    nc.sync.dma_start(
                        out=zt, in_=zT_d[:, ci * KC:(ci + 1) * KC, :])
                    z2 = zp.tile([128, KC, 128], BF16, tag="z2")
                    flat_in = zt.rearrange("c k q -> c (k q)")
                    flat_out = z2.rearrange("c k q -> c (k q)")
                    nsq = KC * 128
                    # DVE is 2x on bf16 tensor_tensor; ACT is 1x — split 60/40
                    cut = (nsq * 3 // 5) & ~1
                    nc.vector.tensor_mul(flat_out[:, 0:cut], flat_in[:, 0:cut],
                                         flat_in[:, 0:cut])
                    nc.scalar.square(flat_out[:, cut:], flat_in[:, cut:])
                    psS = psSp.tile([128, 2 * KC], F32, tag="ss")
                    for half in range(KC // 16):
                        ps = psB.tile([128, 272], F32, tag="zps")
                        for j in range(16):
                            kk = half * 16 + j
                            nc.tensor.matmul(ps[:, j * 17:(j + 1) * 17],
                                             zt[:, kk, :], wext[:, 0:17],
                                             start=True, stop=True)
                            nc.tensor.matmul(psS[:, 2 * kk:2 * kk + 2],
                                             z2[:, kk, :], wext[:, 16:18],
                                             start=True, stop=True)
                        kb = ci * KC + half * 16
                        raw3 = ps[:, 0:272].rearrange("p (k h) -> p k h", h=17)
                        # raw head outputs -> zbraw[q, h, k-slice] (bf16)
                        dst = bass.AP(tensor=zb.tensor,
                                      offset=zb.offset + kb,
                                      ap=[list(zb.ap[0]), [N, H], [1, 16]])
                        src = bass.AP(tensor=ps.tensor, offset=ps.offset,
                                      ap=[list(ps.ap[0]), [1, H], [17, 16]])
                        nc.vector.tensor_copy(dst, src)
                        # mean column -> muraw[q, k-slice]
                        nc.scalar.copy(muraw[:, kb:kb + 16], raw3[:, :, 16])
                    # z^2 sums -> ssraw[q, k-chunk]
                    ss_src = bass.AP(tensor=psS.tensor, offset=psS.offset,
                                     ap=[list(psS.ap[0]), [2, KC]])
                    nc.scalar.copy(ssraw[:, ci * KC:(ci + 1) * KC], ss_src)

                # ---- batched LN finalization, two k-halves so phase C
                # ---- can start injecting as soon as half 0 is normalized
                NH = N // 2
                for hf in range(2):
                    ks = slice(hf * NH, (hf + 1) * NH)
                    mu_s = muraw[:, ks]
                    nc.scalar.mul(mu_s, mu_s, 1.0 / DZ)
                    nc.vector.tensor_mul(mu_s, mu_s, mu_s)  # now mu^2
                    nc.vector.scalar_tensor_tensor(
                        out=ssraw[:, ks], in0=ssraw[:, ks], scalar=1.0 / DZ,
                        in1=mu_s, op0=OP.mult, op1=OP.subtract)  # now var
                    nc.scalar.activation(out=ssraw[:, ks], in_=ssraw[:, ks],
                                         func=AF.Sqrt, bias=eps_t, scale=1.0)
                    with nc.allow_low_precision(reason="alpha ~O(1); bf16 ample"):
                        nc.vector.reciprocal(alpha[:, ks], ssraw[:, ks])
                    # zb *= alpha (broadcast over h), in place.  Done in
                    # 4-head groups so head 0's phase-C injection unblocks
                    # after ~1/4 of the normalize instead of all of it.
                    for hg in range(4):
                        alpha_b = bass.AP(tensor=alpha.tensor,
                                          offset=alpha.offset + hf * NH,
                                          ap=[list(alpha.ap[0]), [0, 4], [1, NH]])
                        zb_g = bass.AP(tensor=zb.tensor,
                                       offset=zb.offset + hf * NH + hg * 4 * N,
                                       ap=[list(zb.ap[0]), [N, 4], [1, NH]])
                        nc.vector.tensor_mul(zb_g, zb_g, alpha_b)

            # ================= phase C: attention =================
            with tc.tile_pool(name="eC", bufs=2) as ep, \
                 tc.tile_pool(name="oC", bufs=1) as op_, \
                 tc.tile_pool(name="psC", bufs=2, space="PSUM") as psC, \
                 tc.tile_pool(name="psO", bufs=1, space="PSUM") as psO:
                o_ps = psO.tile([128, D], F32, name="o_ps")
                for grp in range(8):
                    for h2 in range(2):
                        h = 2 * grp + h2
                        b, r = divmod(h, 4)
                        rs_ = slice(r * 32, (r + 1) * 32)
                        ps_s = psC.tile([128, 1024], F32, tag="sc")
                        for half in range(2):
                            hs = slice(half * 512, (half + 1) * 512)
                            nc.tensor.matmul(ps_s[:, hs], QT[b][rs_, :],
                                             KT[b][rs_, hs],
                                             start=True, stop=False,
                                             tile_position=(r * 32, 0))
                        for half in range(2):
                            hs = slice(half * 512, (half + 1) * 512)
                            nc.tensor.matmul(ps_s[:, hs], identb,
                                             zb[:, h, hs],
                                             start=False, stop=True)
                        e_sb = ep.tile([128, N], BF16, tag="e")
                        nc.scalar.activation(out=e_sb, in_=ps_s, func=AF.Exp,
                                             accum_out=rowsums[:, h:h + 1])
                        # e^T via the DMA xbar (bf16, SBUF->SBUF): frees
                        # ~275 ns/block of PE time and the PSUM->SBUF copy;
                        # the DMA engines are idle in this phase.
                        eT = ep.tile([128, N], BF16, tag="eT")
                        for tt in range(8):
                            nc.sync.dma_start(
                                out=eT[:, tt * 128:(tt + 1) * 128],
                                in_=e_sb[:, tt * 128:(tt + 1) * 128],
                                transpose=True)
                        for tt in range(8):
                            nc.tensor.matmul(
                                o_ps[:, h * 32:(h + 1) * 32],
                                eT[:, tt * 128:(tt + 1) * 128],
                                Vt[tt][:, h * 32:(h + 1) * 32],
                                start=(tt == 0), stop=(tt == 7))
                # normalize + gate + output projection
                wo = [op_.tile([128, D], BF16, name=f"wo{g}") for g in range(4)]
                for g in range(4):
                    nc.sync.dma_start(out=wo[g], in_=wo_d[g * 128:(g + 1) * 128, :])
                rec = op_.tile([128, H], F32, name="rec")
                nc.vector.reciprocal(rec, rowsums)
                rec_b = bass.AP(tensor=rec.tensor, offset=rec.offset,
                                ap=[list(rec.ap[0]), [1, H], [0, HD]])
                onorm = op_.tile([128, D], F32, name="onorm")
                onorm_3d = bass.AP(tensor=onorm.tensor, offset=onorm.offset,
                                   ap=[list(onorm.ap[0]), [HD, H], [1, HD]])
                nc.vector.tensor_mul(onorm_3d, o_ps.rearrange("p (h d) -> p h d", h=H), rec_b)
                og_nat = op_.tile([128, D], BF16, name="og_nat")
                nc.vector.tensor_mul(og_nat, onorm, G_sb)
                ps_tr2 = psC.tile([128, D], BF16, tag="sc")
                for g in range(4):
                    nc.tensor.transpose(ps_tr2[:, g * 128:(g + 1) * 128],
                                        og_nat[:, g * 128:(g + 1) * 128], identb)
                og = [op_.tile([128, 128], BF16, name=f"og{g}") for g in range(4)]
                for g in range(4):
                    nc.scalar.copy(og[g], ps_tr2[:, g * 128:(g + 1) * 128])
                ps_out = psC.tile([128, 512], F32, tag="sc")
                for g in range(4):
                    nc.tensor.matmul(ps_out, og[g], wo[g],
                                     start=(g == 0), stop=(g == 3))
                out_sb = op_.tile([128, D], F32)
                nc.scalar.copy(out_sb, ps_out)
                nc.sync.dma_start(out=out_d[:, :], in_=out_sb)

    nc.compile()
    return nc


def _get_nc():
    global _CACHED
    if _CACHED is None:
        _CACHED = _build()
    return _CACHED


def _prepare_inputs(s, z, norm_s_w, norm_s_b, Wq, bq, Wk, Wv, Wg,
                    z_norm_w, z_norm_b, Wz, Wo):
    s2 = np.asarray(s, np.float32).reshape(N, D)
    z3 = np.asarray(z, np.float32).reshape(N, N, DZ)
    w_s = np.asarray(norm_s_w, np.float32)
    b_s = np.asarray(norm_s_b, np.float32)
    scale = np.float32(HD ** -0.5)
    Wq_f = (w_s[:, None] * np.asarray(Wq, np.float32)) * scale
    bq_f = (np.asarray(bq, np.float32) + b_s @ np.asarray(Wq, np.float32)) * scale
    Wk_f = w_s[:, None] * np.asarray(Wk, np.float32)
    Wv_f = w_s[:, None] * np.asarray(Wv, np.float32)
    bv_f = b_s @ np.asarray(Wv, np.float32)
    Wg_f = w_s[:, None] * np.asarray(Wg, np.float32)
    bg_f = b_s @ np.asarray(Wg, np.float32)
    Wp = np.asarray(z_norm_w, np.float32)[:, None] * np.asarray(Wz, np.float32)
    S = Wp.sum(0)
    Wpp = Wp - np.ones((DZ, 1), np.float32) @ (S[None, :] / DZ)
    Wext = np.ascontiguousarray(
        np.concatenate([Wpp, np.ones((DZ, 1), np.float32),
                        np.zeros((DZ, 1), np.float32)], 1)).astype(ml_dtypes.bfloat16)
    ident = np.eye(128, dtype=np.float32)
    identb = np.eye(128, dtype=ml_dtypes.bfloat16)
    bf = ml_dtypes.bfloat16
    shared = {
        "s_full": s2, "Wq": np.ascontiguousarray(Wq_f).astype(bf),
        "Wk": np.ascontiguousarray(Wk_f).astype(bf),
        "Wv": np.ascontiguousarray(Wv_f).astype(bf),
        "Wg": np.ascontiguousarray(Wg_f).astype(bf),
        "Wo": np.ascontiguousarray(np.asarray(Wo, np.float32)).astype(bf),
        "bq": np.ascontiguousarray(bq_f),
        "bv": np.ascontiguousarray(bv_f), "bg": np.ascontiguousarray(bg_f),
        "Wext": Wext, "ident": ident, "identb": identb,
    }
    zb16 = z3.astype(ml_dtypes.bfloat16)
    in_maps = []
    for c in range(NC):
        qs = slice(c * NQ, (c + 1) * NQ)
        zTc = np.ascontiguousarray(zb16[qs].transpose(2, 1, 0))
        m = dict(shared)
        m["s_q"] = np.ascontiguousarray(s2[qs])
        m["zT"] = zTc
        in_maps.append(m)
    return in_maps


def _run(in_maps, trace=False):
    nc = _get_nc()
    return run_bass_kernel_spmd(nc, in_maps, core_ids=list(range(NC)),
                                trace=trace)


def kernel(**inputs):
    in_maps = _prepare_inputs(**inputs)
    res = _run(in_maps, trace=False)
    out = np.concatenate([res.results[c]["out"] for c in range(NC)], 0)
    return out.reshape(B, N, D).astype(np.float32)


# revision 7
# speedup vs baseline: 1.3484x; 1.3484x over previous
"""AttentionPairBias TRN2 kernel — 8-core SPMD, query-row sharding, v2.

Differences vs v1 baseline:
  - z path in bf16: zT input is bf16 (half the DMA bytes), z-plane stationaries
    get fast-weight-load, z^2 computed in bf16 (2x DVE rate on half).
  - z-LN stats accumulated into persistent [q, k] buffers; the
    mean/var/rsqrt chain runs as a handful of [128, 1024] ops instead of
    per-16k micro-op chains.  zb normalization is one big [128, 16k] op.
  - zb raw stored [q, h, k] (bf16) so the per-head score injection streams
    contiguous k.
  - softmax normalization deferred past the AV matmul: exp (unnormalized,
    bf16) -> transpose -> AV -> scale o by 1/rowsum fused with the gate.
  - bk dropped (softmax-invariant).
  - v3: s-path weights/activations in bf16 too, shrinking SBUF so the zT
    prefetch pool coexists with phase A -- the 33 MB z stream starts at t=0
    instead of waiting ~55 us for phase-A pools to free.  zb finalize split
    into k-halves so phase-C injection starts before the full tail is done.
"""
import sys, os
sys.path.insert(0, "/opt/trn_rl_repo")
import numpy as np
import ml_dtypes

import concourse.bass as bass
import concourse.bacc as bacc
import concourse.mybir as mybir
import concourse.tile as tile
from concourse.bass_utils import run_bass_kernel_spmd

F32 = mybir.dt.float32
F32R = mybir.dt.float32r
BF16 = mybir.dt.bfloat16
AF = mybir.ActivationFunctionType
OP = mybir.AluOpType

B, N, H, HD, D, DZ = 1, 1024, 16, 32, 512, 128
NC = 8
NQ = N // NC          # 128 q rows per core
KC = 64               # k's per DMA chunk: 2 MB bf16 per DMA (~84% of peak
                      # vs 78% at 1 MB per the HW-measured DMA table; the cost
                      # model prices DMA flat and cannot see this)
EPS = 1e-5

_CACHED = None


def _build():
    nc = bacc.Bacc(None, target_bir_lowering=False)

    s_d = nc.dram_tensor("s_full", [N, D], F32, kind="ExternalInput")
    sq_d = nc.dram_tensor("s_q", [NQ, D], F32, kind="ExternalInput")
    zT_d = nc.dram_tensor("zT", [DZ, N, NQ], BF16, kind="ExternalInput")
    wq_d = nc.dram_tensor("Wq", [D, D], BF16, kind="ExternalInput")
    wk_d = nc.dram_tensor("Wk", [D, D], BF16, kind="ExternalInput")
    wv_d = nc.dram_tensor("Wv", [D, D], BF16, kind="ExternalInput")
    wg_d = nc.dram_tensor("Wg", [D, D], BF16, kind="ExternalInput")
    wo_d = nc.dram_tensor("Wo", [D, D], BF16, kind="ExternalInput")
    bq_d = nc.dram_tensor("bq", [D], F32, kind="ExternalInput")
    bv_d = nc.dram_tensor("bv", [D], F32, kind="ExternalInput")
    bg_d = nc.dram_tensor("bg", [D], F32, kind="ExternalInput")
    wext_d = nc.dram_tensor("Wext", [DZ, 18], BF16, kind="ExternalInput")
    id_d = nc.dram_tensor("ident", [128, 128], F32R, kind="ExternalInput")
    idb_d = nc.dram_tensor("identb", [128, 128], BF16, kind="ExternalInput")
    out_d = nc.dram_tensor("out", [NQ, D], F32, kind="ExternalOutput")

    with tile.TileContext(nc) as tc:
        with tc.tile_pool(name="const", bufs=1) as cpool, \
             tc.tile_pool(name="persist", bufs=1) as pp, \
             tc.tile_pool(name="ztp", bufs=4) as ztp:
            ident = cpool.tile([128, 128], F32R)
            nc.sync.dma_start(out=ident, in_=id_d[:, :])
            identb = cpool.tile([128, 128], BF16)
            nc.sync.dma_start(out=identb, in_=idb_d[:, :])
            wext = cpool.tile([DZ, 18], BF16)
            nc.sync.dma_start(out=wext, in_=wext_d[:, :])
            eps_t = cpool.tile([128, 1], F32)
            nc.vector.memset(eps_t, EPS)
            bq_t = cpool.tile([128, 4], F32)
            nc.sync.dma_start(out=bq_t, in_=bq_d[:].rearrange("(b p) -> p b", p=128))
            bg_rep = cpool.tile([128, D], F32)
            bg_ap = bg_d[:]
            nc.gpsimd.dma_start(
                out=bg_rep,
                in_=bass.AP(tensor=bg_ap.tensor, offset=bg_ap.offset,
                            ap=[[0, 128], [1, D]]),
            )
            bv_rep = cpool.tile([128, D], F32)
            bv_ap = bv_d[:]
            nc.gpsimd.dma_start(
                out=bv_rep,
                in_=bass.AP(tensor=bv_ap.tensor, offset=bv_ap.offset,
                            ap=[[0, 128], [1, D]]),
            )

            # ---------- persistent activation storage ----------
            slnT = [pp.tile([128, N], BF16, name=f"slnT{j}") for j in range(4)]
            sqT = pp.tile([128, 4, 128], BF16)        # (d%128, dtile, q)
            KT = [pp.tile([128, N], BF16, name=f"KT{b}") for b in range(4)]
            Vt = [pp.tile([128, D], BF16, name=f"V{t}") for t in range(8)]
            QT = [pp.tile([128, 128], BF16, name=f"QT{b}") for b in range(4)]
            G_sb = pp.tile([128, D], F32, name="G_sb")
            zb = pp.tile([128, H, N], BF16, name="zb")        # (q, h, k)
            muraw = pp.tile([128, N], F32, name="muraw")      # (q, k) sum_c z
            ssraw = pp.tile([128, N], F32, name="ssraw")      # (q, k) sum_c z^2
            alpha = pp.tile([128, N], BF16, name="alpha")
            rowsums = pp.tile([128, H], F32)

            with tc.tile_pool(name="z2B", bufs=2) as zp, \
                 tc.tile_pool(name="psB", bufs=2, space="PSUM") as psB, \
                 tc.tile_pool(name="psS", bufs=2, space="PSUM") as psSp:
              # ================= phase A: s path =================
              with tc.tile_pool(name="sA", bufs=3) as ap_, \
                   tc.tile_pool(name="wA", bufs=1) as wp, \
                   tc.tile_pool(name="psA", bufs=2, space="PSUM") as psA:
                wk = [wp.tile([128, D], BF16, name=f"wk{i}") for i in range(4)]
                wv = [wp.tile([128, D], BF16, name=f"wv{i}") for i in range(4)]
                wq = [wp.tile([128, D], BF16, name=f"wq{i}") for i in range(4)]
                wg = [wp.tile([128, D], BF16, name=f"wg{i}") for i in range(4)]
                for i in range(4):
                    sl = slice(i * 128, (i + 1) * 128)
                    nc.sync.dma_start(out=wk[i], in_=wk_d[sl, :])
                    nc.sync.dma_start(out=wv[i], in_=wv_d[sl, :])
                    nc.sync.dma_start(out=wq[i], in_=wq_d[sl, :])
                    nc.sync.dma_start(out=wg[i], in_=wg_d[sl, :])

                def layernorm_tile(src_ap, tag):
                    st = ap_.tile([128, D], F32, tag="st", name=f"st{tag}")
                    nc.sync.dma_start(out=st, in_=src_ap)
                    stats = ap_.tile([128, 6], F32, tag="stats", name=f"stats{tag}")
                    nc.vector.bn_stats(out=stats, in_=st)
                    mv = ap_.tile([128, 2], F32, tag="mv", name=f"mv{tag}")
                    nc.vector.bn_aggr(out=mv, in_=stats)
                    std = ap_.tile([128, 1], F32, tag="std", name=f"std{tag}")
                    nc.scalar.activation(out=std, in_=mv[:, 1:2], func=AF.Sqrt,
                                         bias=eps_t, scale=1.0)
                    rst = ap_.tile([128, 1], F32, tag="rst", name=f"rst{tag}")
                    nc.vector.reciprocal(rst, std)
                    sln = ap_.tile([128, D], BF16, tag="sln", name=f"sln{tag}")
                    nc.vector.scalar_tensor_tensor(
                        out=sln, in0=st, scalar=mv[:, 0:1],
                        in1=rst.to_broadcast((128, D)),
                        op0=OP.subtract, op1=OP.mult)
                    return sln

                # full-s LN + transpose into slnT
                for t in range(8):
                    sln = layernorm_tile(s_d[t * 128:(t + 1) * 128, :], f"s{t}")
                    ps = psA.tile([128, D], BF16, tag="trA")
                    for j in range(4):
                        nc.tensor.transpose(ps[:, j * 128:(j + 1) * 128],
                                            sln[:, j * 128:(j + 1) * 128], identb)
                    for j in range(4):
                        nc.vector.tensor_copy(slnT[j][:, t * 128:(t + 1) * 128],
                                              ps[:, j * 128:(j + 1) * 128])
                # q-block LN + transpose into sqT
                slnq = layernorm_tile(sq_d[:, :], "q")
                psq = psA.tile([128, D], BF16, tag="trA")
                for j in range(4):
                    nc.tensor.transpose(psq[:, j * 128:(j + 1) * 128],
                                        slnq[:, j * 128:(j + 1) * 128], identb)
                for j in range(4):
                    nc.vector.tensor_copy(sqT[:, j, :], psq[:, j * 128:(j + 1) * 128])

                # KT[b] = (sln @ Wk)^T  -> [hd(128b), tok]   (bk dropped:
                # a per-(q,h) additive constant is softmax-invariant)
                for b in range(4):
                    bs = slice(b * 128, (b + 1) * 128)
                    for half in range(2):
                        hs = slice(half * 512, (half + 1) * 512)
                        ps = psA.tile([128, 512], F32, tag="mmA")
                        for dt_ in range(4):
                            nc.tensor.matmul(ps, wk[dt_][:, bs], slnT[dt_][:, hs],
                                             start=(dt_ == 0), stop=(dt_ == 3))
                        nc.scalar.copy(KT[b][:, hs], ps)
                # V[t] = sln @ Wv + bv  (natural [tok, hd], bf16)
                for t in range(8):
                    ts = slice(t * 128, (t + 1) * 128)
                    ps = psA.tile([128, 512], F32, tag="mmA")
                    for dt_ in range(4):
                        nc.tensor.matmul(ps, slnT[dt_][:, ts], wv[dt_],
                                         start=(dt_ == 0), stop=(dt_ == 3))
                    nc.vector.tensor_add(Vt[t], ps, bv_rep)
                # QT[b] from the q-block
                for b in range(4):
                    bs = slice(b * 128, (b + 1) * 128)
                    psqt = psA.tile([128, 128], F32, tag="mmA")
                    for dt_ in range(4):
                        nc.tensor.matmul(psqt, wq[dt_][:, bs], sqT[:, dt_, :],
                                         start=(dt_ == 0), stop=(dt_ == 3))
                    nc.scalar.activation(out=QT[b], in_=psqt, func=AF.Identity,
                                         bias=bq_t[:, b:b + 1], scale=1.0)
                # G natural [q, D]
                psg = psA.tile([128, D], F32, tag="mmA")
                for dt_ in range(4):
                    nc.tensor.matmul(psg, sqT[:, dt_, :], wg[dt_],
                                     start=(dt_ == 0), stop=(dt_ == 3))
                gsum = ap_.tile([128, D], F32, tag="st", name="gsum")
                nc.vector.tensor_add(gsum, psg, bg_rep)
                nc.scalar.activation(out=G_sb, in_=gsum, func=AF.Sigmoid,
                                     bias=0.0, scale=1.0)

              # ================= phase B: z path =================
              # per k: LDW(z-plane) MM(17 cols: 16 heads + ones) into psB;
              #        LDW(z^2-plane) MM(2 cols: ones, 0) into psS.
              if True:
                for ci in range(N // KC):
                    zt = ztp.tile([128, KC, 128], BF16, tag="zt")
                    nc.sync.dma_start(
                        out=zt, in_=zT_d[:, ci * KC:(ci + 1) * KC, :])
                    z2 = zp.tile([128, KC, 128], BF16, tag="z2")
                    flat_in = zt.rearrange("c k q -> c (k q)")
                    flat_out = z2.rearrange("c k q -> c (k q)")
                    nsq = KC * 128
                    # DVE is 2x on bf16 tensor_tensor; ACT is 1x — split 60/40
                    cut = (nsq * 3 // 5) & ~1
                    nc.vector.tensor_mul(flat_out[:, 0:cut], flat_in[:, 0:cut],
                                         flat_in[:, 0:cut])
                    nc.scalar.square(flat_out[:, cut:], flat_in[:, cut:])
                    psS = psSp.tile([128, 2 * KC], F32, tag="ss")
                    for half in range(KC // 16):
                        ps = psB.tile([128, 272], F32, tag="zps")
                        for j in range(16):
                            kk = half * 16 + j
                            nc.tensor.matmul(ps[:, j * 17:(j + 1) * 17],
                                             zt[:, kk, :], wext[:, 0:17],
                                             start=True, stop=True)
                            nc.tensor.matmul(psS[:, 2 * kk:2 * kk + 2],
                                             z2[:, kk, :], wext[:, 16:18],
                                             start=True, stop=True)
                        kb = ci * KC + half * 16
                        raw3 = ps[:, 0:272].rearrange("p (k h) -> p k h", h=17)
                        # raw head outputs -> zbraw[q, h, k-slice] (bf16)
                        dst = bass.AP(tensor=zb.tensor,
                                      offset=zb.offset + kb,
                                      ap=[list(zb.ap[0]), [N, H], [1, 16]])
                        src = bass.AP(tensor=ps.tensor, offset=ps.offset,
                                      ap=[list(ps.ap[0]), [1, H], [17, 16]])
                        nc.vector.tensor_copy(dst, src)
                        # mean column -> muraw[q, k-slice]
                        nc.scalar.copy(muraw[:, kb:kb + 16], raw3[:, :, 16])
                    # z^2 sums -> ssraw[q, k-chunk]
                    ss_src = bass.AP(tensor=psS.tensor, offset=psS.offset,
                                     ap=[list(psS.ap[0]), [2, KC]])
                    nc.scalar.copy(ssraw[:, ci * KC:(ci + 1) * KC], ss_src)

                # ---- batched LN finalization, two k-halves so phase C
                # ---- can start injecting as soon as half 0 is normalized
                NH = N // 2
                for hf in range(2):
                    ks = slice(hf * NH, (hf + 1) * NH)
                    mu_s = muraw[:, ks]
                    nc.scalar.mul(mu_s, mu_s, 1.0 / DZ)
                    nc.vector.tensor_mul(mu_s, mu_s, mu_s)  # now mu^2
                    nc.vector.scalar_tensor_tensor(
                        out=ssraw[:, ks], in0=ssraw[:, ks], scalar=1.0 / DZ,
                        in1=mu_s, op0=OP.mult, op1=OP.subtract)  # now var
                    nc.scalar.activation(out=ssraw[:, ks], in_=ssraw[:, ks],
                                         func=AF.Sqrt, bias=eps_t, scale=1.0)
                    with nc.allow_low_precision(reason="alpha ~O(1); bf16 ample"):
                        nc.vector.reciprocal(alpha[:, ks], ssraw[:, ks])
                    # zb *= alpha (broadcast over h), in place.  Done in
                    # 4-head groups so head 0's phase-C injection unblocks
                    # after ~1/4 of the normalize instead of all of it.
                    for hg in range(4):
                        alpha_b = bass.AP(tensor=alpha.tensor,
                                          offset=alpha.offset + hf * NH,
                                          ap=[list(alpha.ap[0]), [0, 4], [1, NH]])
                        zb_g = bass.AP(tensor=zb.tensor,
                                       offset=zb.offset + hf * NH + hg * 4 * N,
                                       ap=[list(zb.ap[0]), [N, 4], [1, NH]])
                        nc.vector.tensor_mul(zb_g, zb_g, alpha_b)

            # ================= phase C: attention =================
            with tc.tile_pool(name="eC", bufs=2) as ep, \
                 tc.tile_pool(name="oC", bufs=1) as op_, \
                 tc.tile_pool(name="psC", bufs=2, space="PSUM") as psC, \
                 tc.tile_pool(name="psO", bufs=1, space="PSUM") as psO:
                o_ps = psO.tile([128, D], F32, name="o_ps")
                for grp in range(8):
                    for h2 in range(2):
                        h = 2 * grp + h2
                        b, r = divmod(h, 4)
                        rs_ = slice(r * 32, (r + 1) * 32)
                        ps_s = psC.tile([128, 1024], F32, tag="sc")
                        for half in range(2):
                            hs = slice(half * 512, (half + 1) * 512)
                            nc.tensor.matmul(ps_s[:, hs], QT[b][rs_, :],
                                             KT[b][rs_, hs],
                                             start=True, stop=False,
                                             tile_position=(r * 32, 0))
                        for half in range(2):
                            hs = slice(half * 512, (half + 1) * 512)
                            nc.tensor.matmul(ps_s[:, hs], identb,
                                             zb[:, h, hs],
                                             start=False, stop=True)
                        e_sb = ep.tile([128, N], BF16, tag="e")
                        nc.scalar.activation(out=e_sb, in_=ps_s, func=AF.Exp,
                                             accum_out=rowsums[:, h:h + 1])
                        # e^T via the DMA xbar (bf16, SBUF->SBUF): frees
                        # ~275 ns/block of PE time and the PSUM->SBUF copy;
                        # the DMA engines are idle in this phase.
                        eT = ep.tile([128, N], BF16, tag="eT")
                        for tt in range(8):
                            nc.sync.dma_start(
                                out=eT[:, tt * 128:(tt + 1) * 128],
                                in_=e_sb[:, tt * 128:(tt + 1) * 128],
                                transpose=True)
                        for tt in range(8):
                            nc.tensor.matmul(
                                o_ps[:, h * 32:(h + 1) * 32],
                                eT[:, tt * 128:(tt + 1) * 128],
                                Vt[tt][:, h * 32:(h + 1) * 32],
                                start=(tt == 0), stop=(tt == 7))
                # normalize + gate + output projection
                wo = [op_.tile([128, D], BF16, name=f"wo{g}") for g in range(4)]
                for g in range(4):
                    nc.sync.dma_start(out=wo[g], in_=wo_d[g * 128:(g + 1) * 128, :])
                rec = op_.tile([128, H], F32, name="rec")
                nc.vector.reciprocal(rec, rowsums)
                rec_b = bass.AP(tensor=rec.tensor, offset=rec.offset,
                                ap=[list(rec.ap[0]), [1, H], [0, HD]])
                onorm = op_.tile([128, D], F32, name="onorm")
                onorm_3d = bass.AP(tensor=onorm.tensor, offset=onorm.offset,
                                   ap=[list(onorm.ap[0]), [HD, H], [1, HD]])
                nc.vector.tensor_mul(onorm_3d, o_ps.rearrange("p (h d) -> p h d", h=H), rec_b)
                og_nat = op_.tile([128, D], BF16, name="og_nat")
                nc.vector.tensor_mul(og_nat, onorm, G_sb)
                ps_tr2 = psC.tile([128, D], BF16, tag="sc")
                for g in range(4):
                    nc.tensor.transpose(ps_tr2[:, g * 128:(g + 1) * 128],
                                        og_nat[:, g * 128:(g + 1) * 128], identb)
                og = [op_.tile([128, 128], BF16, name=f"og{g}") for g in range(4)]
                for g in range(4):
                    nc.scalar.copy(og[g], ps_tr2[:, g * 128:(g + 1) * 128])
                ps_out = psC.tile([128, 512], F32, tag="sc")
                for g in range(4):
                    nc.tensor.matmul(ps_out, og[g], wo[g],
                                     start=(g == 0), stop=(g == 3))
                out_sb = op_.tile([128, D], F32)
                nc.scalar.copy(out_sb, ps_out)
                nc.sync.dma_start(out=out_d[:, :], in_=out_sb)

    nc.compile()
    return nc


def _get_nc():
    global _CACHED
    if _CACHED is None:
        _CACHED = _build()
    return _CACHED


def _prepare_inputs(s, z, norm_s_w, norm_s_b, Wq, bq, Wk, Wv, Wg,
                    z_norm_w, z_norm_b, Wz, Wo):
    s2 = np.asarray(s, np.float32).reshape(N, D)
    z3 = np.asarray(z, np.float32).reshape(N, N, DZ)
    w_s = np.asarray(norm_s_w, np.float32)
    b_s = np.asarray(norm_s_b, np.float32)
    scale = np.float32(HD ** -0.5)
    Wq_f = (w_s[:, None] * np.asarray(Wq, np.float32)) * scale
    bq_f = (np.asarray(bq, np.float32) + b_s @ np.asarray(Wq, np.float32)) * scale
    Wk_f = w_s[:, None] * np.asarray(Wk, np.float32)
    Wv_f = w_s[:, None] * np.asarray(Wv, np.float32)
    bv_f = b_s @ np.asarray(Wv, np.float32)
    Wg_f = w_s[:, None] * np.asarray(Wg, np.float32)
    bg_f = b_s @ np.asarray(Wg, np.float32)
    Wp = np.asarray(z_norm_w, np.float32)[:, None] * np.asarray(Wz, np.float32)
    S = Wp.sum(0)
    Wpp = Wp - np.ones((DZ, 1), np.float32) @ (S[None, :] / DZ)
    Wext = np.ascontiguousarray(
        np.concatenate([Wpp, np.ones((DZ, 1), np.float32),
                        np.zeros((DZ, 1), np.float32)], 1)).astype(ml_dtypes.bfloat16)
    ident = np.eye(128, dtype=np.float32)
    identb = np.eye(128, dtype=ml_dtypes.bfloat16)
    bf = ml_dtypes.bfloat16
    shared = {
        "s_full": s2, "Wq": np.ascontiguousarray(Wq_f).astype(bf),
        "Wk": np.ascontiguousarray(Wk_f).astype(bf),
        "Wv": np.ascontiguousarray(Wv_f).astype(bf),
        "Wg": np.ascontiguousarray(Wg_f).astype(bf),
        "Wo": np.ascontiguousarray(np.asarray(Wo, np.float32)).astype(bf),
        "bq": np.ascontiguousarray(bq_f),
        "bv": np.ascontiguousarray(bv_f), "bg": np.ascontiguousarray(bg_f),
        "Wext": Wext, "ident": ident, "identb": identb,
    }
    zb16 = z3.astype(ml_dtypes.bfloat16)
    in_maps = []
    for c in range(NC):
        qs = slice(c * NQ, (c + 1) * NQ)
        zTc = np.ascontiguousarray(zb16[qs].transpose(2, 1, 0))
        m = dict(shared)
        m["s_q"] = np.ascontiguousarray(s2[qs])
        m["zT"] = zTc
        in_maps.append(m)
    return in_maps


def _run(in_maps, trace=False):
    nc = _get_nc()
    return run_bass_kernel_spmd(nc, in_maps, core_ids=list(range(NC)),
                                trace=trace)


def kernel(**inputs):
    in_maps = _prepare_inputs(**inputs)
    res = _run(in_maps, trace=False)
    out = np.concatenate([res.results[c]["out"] for c in range(NC)], 0)
    return out.reshape(B, N, D).astype(np.float32)
